# revision 7
# baseline (speedup 1.0000x reference)
"""Fused AttentionLocal kernel for 8 Trainium2 NeuronCores.

Pipeline per batch element b (data-parallel over batch):
  h  = conv7x7_dil2(x)                       [256, 32, 32]
  k  = softmax_ch(BN2(conv1x1(BN1(h))))      [1024, 32, 32]
  y[p, c] = sum_n k[n, p] * x[c, n] / sum_n k[n, p]   (n, p = flattened 32x32)

BN folding (training-mode BN, global batch stats):
  BN1 folds into conv2's weights:   k_raw = (W2 * a1) @ h_raw + cst
  BN2+softmax folds into the exp:   exp(a2 * k_raw + bias2)
  Global stats come from ONE AllReduce of [sum(h), sum(h^2), G = h @ h^T];
  E[k^2] is recovered analytically from the Gram matrix G.
"""

import itertools

import numpy as np

import concourse.bass as bass
import concourse.tile as tile
from concourse import bacc, mybir
from concourse.masks import make_identity

F32 = mybir.dt.float32
F32R = mybir.dt.float32r
AF = mybir.ActivationFunctionType
ALU = mybir.AluOpType
AX = mybir.AxisListType
EPS = 1e-5

N_CORES = 8
B_GLOBAL = 64
C = 256
HW = 1024
W2OUT = 1024

# tap order: (3,3) first so the start=True matmul covers the full PSUM bank
TAPS_ALL = [(3, 3)] + [t for t in itertools.product(range(7), range(7)) if t != (3, 3)]
TAP_GROUPS = []
_i = 0
for _g in (10, 10, 10, 10, 9):
    TAP_GROUPS.append(list(range(_i, _i + _g)))
    _i += _g


def build_body(tc, aps, n_cores, b_loc, total_batch):
    nc = tc.nc
    P_TOT = float(total_batch * HW)
    x_ap = aps["x"]
    w1t_ap = aps["w1t"]
    w2t_ap = aps["w2t"]
    out_ap = aps["out"]

    sbsz = min(4, b_loc)
    n_sb = (b_loc + sbsz - 1) // sbsz

    import contextlib
    ctx = contextlib.ExitStack()
    with ctx:
        persist = ctx.enter_context(tc.tile_pool(name="persist", bufs=1))
        dram = ctx.enter_context(tc.tile_pool(name="dram", bufs=1, space="DRAM"))

        # ---------------- prologue: constants + params ----------------
        ident = persist.tile([128, 128], F32, tag="ident", name="ident")
        make_identity(nc, ident[:])
        ones_f32 = persist.tile([128, 2], F32, tag="ones_f32", name="ones_f32")
        nc.gpsimd.memset(ones_f32[:], 1.0)
        ones_col = persist.tile([128, 1], F32R, tag="ones_col", name="ones_col")
        nc.vector.tensor_copy(ones_col[:], ones_f32[:, 0:1])
        zero192 = persist.tile([128, 192], F32, tag="zero192", name="zero192")
        nc.gpsimd.memset(zero192[:], 0.0)

        def row_tile(name, src_1d, n):
            t = persist.tile([1, n], F32, tag=name, name=name)
            nc.sync.dma_start(t[:], src_1d.rearrange("(o n) -> o n", o=1))
            return t

        g1row = row_tile("g1row", aps["bn1g"], C)
        b1row = row_tile("b1row", aps["bn1b"], C)
        g2row = row_tile("g2row", aps["bn2g"], W2OUT)
        b2row = row_tile("b2row", aps["bn2b"], W2OUT)
        cb2row = row_tile("cb2row", aps["cb2"], W2OUT)

        w2t_t = []
        for cc in range(2):
            t = persist.tile([128, W2OUT], F32R, tag=f"w2t{cc}", name=f"w2t{cc}")
            nc.sync.dma_start(t[:], w2t_ap[cc * 128:(cc + 1) * 128, :].bitcast(F32R))
            w2t_t.append(t)

        h_tiles = {}
        for b in range(b_loc):
            for oc in range(2):
                h_tiles[(b, oc)] = persist.tile([128, HW], F32R, tag=f"h{b}_{oc}", name=f"h{b}_{oc}")

        s_acc = [persist.tile([128, b_loc], F32, tag=f"sacc{oc}", name=f"sacc{oc}") for oc in range(2)]
        q_acc = [persist.tile([128, b_loc], F32, tag=f"qacc{oc}", name=f"qacc{oc}") for oc in range(2)]
        s_col = [persist.tile([128, 1], F32, tag=f"scol{oc}", name=f"scol{oc}") for oc in range(2)]
        q_col = [persist.tile([128, 1], F32, tag=f"qcol{oc}", name=f"qcol{oc}") for oc in range(2)]
        bn1pp = [persist.tile([128, 3], F32R, tag=f"bn1pp{oc}", name=f"bn1pp{oc}") for oc in range(2)]
        bn2pp = persist.tile([128, 16], F32, tag="bn2pp", name="bn2pp")

        stats_in = dram.tile([C + 2, C], F32, tag="stats_in", name="stats_in")
        stats_out = dram.tile([C + 2, C], F32, tag="stats_out", name="stats_out")
        bn1_bounce = dram.tile([2, C], F32, tag="bn1_bounce", name="bn1_bounce")
        bn2_bounce = dram.tile([2, W2OUT], F32, tag="bn2_bounce", name="bn2_bounce")

        # ---------------- phase 1: conv1 (dilated 7x7 as 49 shifted matmuls) ----
        with tc.tile_pool(name="convsb", bufs=1) as convsb, \
             tc.tile_pool(name="convps", bufs=8, space="PSUM") as convps:
            for sb in range(n_sb):
                bls = list(range(sb * sbsz, min((sb + 1) * sbsz, b_loc)))
                cps = {}
                for b in bls:
                    for cc in range(2):
                        cp = convsb.tile([128, 32 * 44], F32R, tag=f"colpad{(b % sbsz) * 2 + cc}", name=f"colpad{(b % sbsz) * 2 + cc}")
                        v = cp[:].rearrange("p (r c) -> p r c", c=44)
                        zsrc = zero192[:].rearrange("p (r c) -> p r c", c=6)
                        nc.vector.tensor_copy(v[:, :, 0:6], zsrc)
                        nc.vector.tensor_copy(v[:, :, 38:44], zsrc)
                        nc.sync.dma_start(
                            v[:, :, 6:38],
                            x_ap[b, cc * 128:(cc + 1) * 128, :]
                            .rearrange("p (r c) -> p r c", c=32).bitcast(F32R))
                        cps[(b, cc)] = v
                for co in range(2):
                    pss = {}
                    for b in bls:
                        for hf in range(2):
                            pss[(b, hf)] = convps.tile([128, 512], F32, tag="convps", name="convps")
                    for gi, group in enumerate(TAP_GROUPS):
                        g0 = group[0]
                        wts = []
                        for cc in range(2):
                            wt_ = convsb.tile([128, len(group) * 128], F32R, tag=f"w1c{cc}", name=f"w1c{cc}")
                            nc.sync.dma_start(
                                wt_[:].rearrange("p (t k) -> p t k", k=128),
                                w1t_ap[g0:g0 + len(group),
                                       cc * 128:(cc + 1) * 128,
                                       co * 128:(co + 1) * 128]
                                .rearrange("t p k -> p t k").bitcast(F32R))
                            wts.append(wt_)
                        for b in bls:
                            for hf in range(2):
                                for ti, tap in enumerate(group):
                                    kh, kw = TAPS_ALL[tap]
                                    dy, dx = 2 * kh - 6, 2 * kw - 6
                                    r0 = max(hf * 16, -dy)
                                    r1 = min(hf * 16 + 16, 32 - dy)
                                    if r1 <= r0:
                                        continue
                                    out_v = pss[(b, hf)][:].rearrange(
                                        "p (r c) -> p r c", c=32)[:, r0 - hf * 16:r1 - hf * 16, :]
                                    for cc in range(2):
                                        first = (gi == 0 and ti == 0 and cc == 0)
                                        last = (gi == len(TAP_GROUPS) - 1
                                                and ti == len(group) - 1 and cc == 1)
                                        nc.tensor.matmul(
                                            out_v,
                                            wts[cc][:, ti * 128:(ti + 1) * 128],
                                            cps[(b, cc)][:, r0 + dy:r1 + dy, 6 + dx:6 + dx + 32],
                                            start=first, stop=last, skip_group_check=True)
                    for b in bls:
                        for hf in range(2):
                            nc.vector.tensor_copy(
                                h_tiles[(b, co)][:, hf * 512:(hf + 1) * 512], pss[(b, hf)][:])

        # ---------------- phase 2: stats + Gram + allreduce ---------------------
        if True:
            with tc.tile_pool(name="gsb", bufs=1) as gsb_pool, \
                 tc.tile_pool(name="gscr", bufs=2) as gscr, \
                 tc.tile_pool(name="hTp", bufs=3) as hTp, \
                 tc.tile_pool(name="tps", bufs=4, space="PSUM") as tps, \
                 tc.tile_pool(name="gps", bufs=1, space="PSUM") as gps_pool:

                # per-channel sum and sum-of-squares of h
                for b in range(b_loc):
                    for oc in range(2):
                        nc.vector.reduce_sum(
                            s_acc[oc][:, b:b + 1], h_tiles[(b, oc)][:].bitcast(F32), axis=AX.X)
                        scr = gscr.tile([128, HW], F32, tag="ttr", name="ttr")
                        nc.scalar.activation(
                            scr[:], h_tiles[(b, oc)][:].bitcast(F32), AF.Square,
                            accum_out=q_acc[oc][:, b:b + 1])
                for oc in range(2):
                    nc.vector.reduce_sum(s_col[oc][:], s_acc[oc][:], axis=AX.X)
                    nc.vector.reduce_sum(q_col[oc][:], q_acc[oc][:], axis=AX.X)

                # G = h @ h^T via PE transposes of h
                gps = [gps_pool.tile([128, C], F32, tag=f"gps{oc}", name=f"gps{oc}") for oc in range(2)]
                for b in range(b_loc):
                    for j in range(8):
                        hT = hTp.tile([128, C], F32R, tag="hT", name="hT")
                        for oc in range(2):
                            tp = tps.tile([128, 128], F32, tag="tp", name="tp")
                            nc.tensor.matmul(
                                tp[:], h_tiles[(b, oc)][:, j * 128:(j + 1) * 128].bitcast(F32),
                                ident[:], is_transpose=True)
                            nc.vector.tensor_copy(hT[:, oc * 128:(oc + 1) * 128], tp[:])
                        for oc in range(2):
                            first = (b == 0 and j == 0)
                            last = (b == b_loc - 1 and j == 7)
                            nc.tensor.matmul(
                                gps[oc][:], hT[:, oc * 128:(oc + 1) * 128], hT[:],
                                start=first, stop=last, skip_group_check=True)
                for oc in range(2):
                    gsb = gsb_pool.tile([128, C], F32, tag=f"gsb{oc}", name=f"gsb{oc}")
                    nc.vector.tensor_copy(gsb[:], gps[oc][:])
                    nc.sync.dma_start(stats_in[oc * 128:(oc + 1) * 128, :], gsb[:])
                    nc.sync.dma_start(
                        stats_in[C:C + 1, oc * 128:(oc + 1) * 128].rearrange("o p -> p o"),
                        s_col[oc][:])
                    nc.sync.dma_start(
                        stats_in[C + 1:C + 2, oc * 128:(oc + 1) * 128].rearrange("o p -> p o"),
                        q_col[oc][:])

                nc.gpsimd.collective_compute(
                    "AllReduce", ALU.add,
                    replica_groups=[list(range(n_cores))],
                    ins=[stats_in.opt()],
                    outs=[stats_out.opt()])

            # ---------------- phase 3: BN constants from global stats ----------
            with tc.tile_pool(name="rows", bufs=1) as rows, \
                 tc.tile_pool(name="prodp", bufs=2) as prodp, \
                 tc.tile_pool(name="rowps", bufs=2, space="PSUM") as rowps, \
                 tc.tile_pool(name="m1ps", bufs=2, space="PSUM") as m1ps:

                def rt(name, n=C):
                    return rows.tile([1, n], F32, tag=name, name=name)

                g_glob = []
                for oc in range(2):
                    gg = rows.tile([128, C], F32R, tag=f"gglob{oc}", name=f"gglob{oc}")
                    nc.sync.dma_start(gg[:], stats_out[oc * 128:(oc + 1) * 128, :].bitcast(F32R))
                    g_glob.append(gg)
                s_row = rt("s_row")
                nc.sync.dma_start(s_row[:], stats_out[C:C + 1, :])
                q_row = rt("q_row")
                nc.sync.dma_start(q_row[:], stats_out[C + 1:C + 2, :])

                meanh = rt("meanh")
                nc.vector.tensor_scalar_mul(meanh[:], s_row[:], 1.0 / P_TOT)
                msq = rt("msq")
                nc.vector.tensor_mul(msq[:], meanh[:], meanh[:])
                var1 = rt("var1")
                nc.vector.tensor_scalar_mul(var1[:], q_row[:], 1.0 / P_TOT)
                nc.vector.tensor_sub(var1[:], var1[:], msq[:])
                nc.vector.tensor_scalar_add(var1[:], var1[:], EPS)
                rec1 = rt("rec1")
                nc.vector.reciprocal(rec1[:], var1[:])
                rsq1 = rt("rsq1")
                nc.scalar.activation(rsq1[:], rec1[:], AF.Sqrt)
                a1row = rt("a1row")
                nc.vector.tensor_mul(a1row[:], rsq1[:], g1row[:])
                tmp1 = rt("tmp1")
                nc.vector.tensor_mul(tmp1[:], a1row[:], meanh[:])
                c1srow = rt("c1srow")
                nc.vector.tensor_sub(c1srow[:], b1row[:], tmp1[:])

                nc.sync.dma_start(bn1_bounce[0:1, :], a1row[:])
                nc.sync.dma_start(bn1_bounce[1:2, :], c1srow[:])
                for oc in range(2):
                    nc.sync.dma_start(
                        bn1pp[oc][:, 0:2],
                        bn1_bounce[:, oc * 128:(oc + 1) * 128]
                        .rearrange("r p -> p r").bitcast(F32R))
                    nc.sync.dma_start(
                        bn1pp[oc][:, 2:3],
                        stats_out[C:C + 1, oc * 128:(oc + 1) * 128]
                        .rearrange("o p -> p o").bitcast(F32R))

                # cst[n] = sum_c W2[n,c] * c1s[c] + conv2_b[n]  (unscaled W2)
                cstrow = rt("cstrow", W2OUT)
                for nh in range(2):
                    cp_ = rowps.tile([1, 512], F32, tag="rowps", name="rowps")
                    for oc in range(2):
                        nc.tensor.matmul(
                            cp_[:], bn1pp[oc][:, 1:2], w2t_t[oc][:, nh * 512:(nh + 1) * 512],
                            start=(oc == 0), stop=(oc == 1), skip_group_check=True)
                    nc.vector.tensor_add(
                        cstrow[:, nh * 512:(nh + 1) * 512], cp_[0:1, :],
                        cb2row[:, nh * 512:(nh + 1) * 512])

                # scale W2T in place by a1 (per input channel)
                for oc in range(2):
                    nc.vector.tensor_scalar_mul(
                        w2t_t[oc][:], w2t_t[oc][:], bn1pp[oc][:, 0:1].bitcast(F32))

                # r1[n] = sum_c W2'[n,c] * s[c]  (scaled W2, unscaled s)
                r1row = rt("r1row", W2OUT)
                for nh in range(2):
                    rp_ = rowps.tile([1, 512], F32, tag="rowps", name="rowps")
                    for oc in range(2):
                        nc.tensor.matmul(
                            rp_[:], bn1pp[oc][:, 2:3], w2t_t[oc][:, nh * 512:(nh + 1) * 512],
                            start=(oc == 0), stop=(oc == 1), skip_group_check=True)
                    nc.vector.tensor_copy(r1row[:, nh * 512:(nh + 1) * 512], rp_[0:1, :])

                # M1 = G @ W2'^T ; e[n] = sum_c W2'[n,c] * M1[c,n]
                M1 = [rows.tile([128, W2OUT], F32R, tag=f"M1_{oc}", name=f"M1_{oc}") for oc in range(2)]
                for occ in range(2):
                    for nh in range(2):
                        mp = m1ps.tile([128, 512], F32, tag="m1ps", name="m1ps")
                        for dd in range(2):
                            nc.tensor.matmul(
                                mp[:], g_glob[dd][:, occ * 128:(occ + 1) * 128],
                                w2t_t[dd][:, nh * 512:(nh + 1) * 512],
                                start=(dd == 0), stop=(dd == 1), skip_group_check=True)
                        nc.vector.tensor_copy(M1[occ][:, nh * 512:(nh + 1) * 512], mp[:])
                erow = rt("erow", W2OUT)
                prods = []
                for oc in range(2):
                    pr = prodp.tile([128, W2OUT], F32R, tag="prod", name="prod")
                    nc.vector.tensor_mul(pr[:], w2t_t[oc][:].bitcast(F32), M1[oc][:].bitcast(F32))
                    prods.append(pr)
                for nh in range(2):
                    ep_ = rowps.tile([1, 512], F32, tag="rowps", name="rowps")
                    for oc in range(2):
                        nc.tensor.matmul(
                            ep_[:], ones_col[:], prods[oc][:, nh * 512:(nh + 1) * 512],
                            start=(oc == 0), stop=(oc == 1), skip_group_check=True)
                    nc.vector.tensor_copy(erow[:, nh * 512:(nh + 1) * 512], ep_[0:1, :])

                # BN2 rows
                mkrow = rt("mkrow", W2OUT)
                nc.vector.tensor_scalar_mul(mkrow[:], r1row[:], 1.0 / P_TOT)
                nc.vector.tensor_add(mkrow[:], mkrow[:], cstrow[:])
                t1 = rt("t1", W2OUT)
                nc.vector.tensor_mul(t1[:], cstrow[:], r1row[:])
                nc.vector.tensor_scalar_mul(t1[:], t1[:], 2.0 / P_TOT)
                t2 = rt("t2", W2OUT)
                nc.vector.tensor_mul(t2[:], cstrow[:], cstrow[:])
                ek2 = rt("ek2", W2OUT)
                nc.vector.tensor_scalar_mul(ek2[:], erow[:], 1.0 / P_TOT)
                nc.vector.tensor_add(ek2[:], ek2[:], t1[:])
                nc.vector.tensor_add(ek2[:], ek2[:], t2[:])
                mks = rt("mks", W2OUT)
                nc.vector.tensor_mul(mks[:], mkrow[:], mkrow[:])
                var2 = rt("var2", W2OUT)
                nc.vector.tensor_sub(var2[:], ek2[:], mks[:])
                nc.vector.tensor_scalar_add(var2[:], var2[:], EPS)
                rec2 = rt("rec2", W2OUT)
                nc.vector.reciprocal(rec2[:], var2[:])
                rsq2 = rt("rsq2", W2OUT)
                nc.scalar.activation(rsq2[:], rec2[:], AF.Sqrt)
                a2row = rt("a2row", W2OUT)
                nc.vector.tensor_mul(a2row[:], rsq2[:], g2row[:])
                t3 = rt("t3", W2OUT)
                nc.vector.tensor_scalar_mul(t3[:], r1row[:], 1.0 / P_TOT)
                nc.vector.tensor_mul(t3[:], a2row[:], t3[:])
                bias2row = rt("bias2row", W2OUT)
                nc.vector.tensor_sub(bias2row[:], b2row[:], t3[:])

                nc.sync.dma_start(bn2_bounce[0:1, :], a2row[:])
                nc.sync.dma_start(bn2_bounce[1:2, :], bias2row[:])
                nc.sync.dma_start(
                    bn2pp[:],
                    bn2_bounce[:].rearrange("w (k p) -> p (w k)", p=128))

            # ---------------- phase 4: x transposes + conv2 + exp + attention ---
            with tc.tile_pool(name="xta", bufs=1) as xta_pool, \
                 tc.tile_pool(name="xinp", bufs=2) as xinp, \
                 tc.tile_pool(name="kexp", bufs=12) as kexp_pool, \
                 tc.tile_pool(name="outp", bufs=4) as outp, \
                 tc.tile_pool(name="recp", bufs=4) as recp, \
                 tc.tile_pool(name="tps2", bufs=2, space="PSUM") as tps2, \
                 tc.tile_pool(name="c2ps", bufs=3, space="PSUM") as c2ps, \
                 tc.tile_pool(name="aps", bufs=3, space="PSUM") as aps_pool:
                # xTa[b][j] = [x[b]^T slice | ones] for the attention matmul
                xta = {}
                for b in range(b_loc):
                    xins = []
                    for cc in range(2):
                        xin = xinp.tile([128, HW], F32R, tag=f"xin{cc}", name=f"xin{cc}")
                        nc.sync.dma_start(
                            xin[:], x_ap[b, cc * 128:(cc + 1) * 128, :].bitcast(F32R))
                        xins.append(xin)
                    for j in range(8):
                        xt_ = xta_pool.tile([128, 258], F32R, tag=f"xta{b}_{j}", name=f"xta{b}_{j}")
                        for cc in range(2):
                            tp = tps2.tile([128, 128], F32, tag="tp2", name="tp2")
                            nc.tensor.matmul(
                                tp[:], xins[cc][:, j * 128:(j + 1) * 128].bitcast(F32),
                                ident[:], is_transpose=True)
                            nc.vector.tensor_copy(xt_[:, cc * 128:(cc + 1) * 128], tp[:])
                        nc.vector.tensor_copy(xt_[:, 256:258], ones_f32[:])
                        xta[(b, j)] = xt_
                for b in range(b_loc):
                    for hf in range(2):
                        ke = []
                        for j in range(8):
                            cp_ = c2ps.tile([128, 512], F32, tag="c2ps", name="c2ps")
                            for cc in range(2):
                                nc.tensor.matmul(
                                    cp_[:], w2t_t[cc][:, j * 128:(j + 1) * 128],
                                    h_tiles[(b, cc)][:, hf * 512:(hf + 1) * 512],
                                    start=(cc == 0), stop=(cc == 1), skip_group_check=True)
                            ket = kexp_pool.tile([128, 512], F32R, tag="ke", name="ke")
                            nc.scalar.activation(
                                ket[:], cp_[:], AF.Exp,
                                bias=bn2pp[:, 8 + j:9 + j], scale=bn2pp[:, j:j + 1])
                            ke.append(ket)
                        for pc in range(4):
                            ap_ = aps_pool.tile([128, 258], F32, tag="aps", name="aps")
                            for j in range(8):
                                nc.tensor.matmul(
                                    ap_[:], ke[j][:, pc * 128:(pc + 1) * 128], xta[(b, j)][:],
                                    start=(j == 0), stop=(j == 7), skip_group_check=True)
                            rec = recp.tile([128, 1], F32, tag="rec", name="rec")
                            nc.vector.reciprocal(rec[:], ap_[:, 256:257])
                            osb = outp.tile([128, C], F32, tag="osb", name="osb")
                            nc.vector.tensor_scalar_mul(osb[:], ap_[:, 0:256], rec[:])
                            r0 = hf * 512 + pc * 128
                            nc.sync.dma_start(out_ap[b, r0:r0 + 128, :], osb[:])


def build(n_cores=N_CORES, b_loc=B_GLOBAL // N_CORES, total_batch=B_GLOBAL):
    nc = bacc.Bacc("TRN2", target_bir_lowering=False, debug=False, num_devices=n_cores)
    aps = {
        "x": nc.dram_tensor("x", [b_loc, C, HW], F32, kind="ExternalInput").ap(),
        "w1t": nc.dram_tensor("w1t", [49, C, C], F32, kind="ExternalInput").ap(),
        "w2t": nc.dram_tensor("w2t", [C, W2OUT], F32, kind="ExternalInput").ap(),
        "bn1g": nc.dram_tensor("bn1g", [C], F32, kind="ExternalInput").ap(),
        "bn1b": nc.dram_tensor("bn1b", [C], F32, kind="ExternalInput").ap(),
        "bn2g": nc.dram_tensor("bn2g", [W2OUT], F32, kind="ExternalInput").ap(),
        "bn2b": nc.dram_tensor("bn2b", [W2OUT], F32, kind="ExternalInput").ap(),
        "cb2": nc.dram_tensor("cb2", [W2OUT], F32, kind="ExternalInput").ap(),
        "out": nc.dram_tensor("out", [b_loc, HW, C], F32, kind="ExternalOutput").ap(),
    }
    with tile.TileContext(nc) as tc:
        build_body(tc, aps, n_cores, b_loc, total_batch)
    nc.compile()
    return nc


_CACHE = {}


def _prep_in_maps(inputs, n_cores, b_loc):
    perm = [kh * 7 + kw for (kh, kw) in TAPS_ALL]
    w1t = np.ascontiguousarray(
        np.asarray(inputs["conv1_w"], np.float32).transpose(2, 3, 1, 0).reshape(49, C, C)[perm])
    w2t = np.ascontiguousarray(np.asarray(inputs["conv2_w"], np.float32)[:, :, 0, 0].T)
    shared = {
        "w1t": w1t,
        "w2t": w2t,
        "bn1g": np.asarray(inputs["bn1_g"], np.float32),
        "bn1b": np.asarray(inputs["bn1_b"], np.float32),
        "bn2g": np.asarray(inputs["bn2_g"], np.float32),
        "bn2b": np.asarray(inputs["bn2_b"], np.float32),
        "cb2": np.asarray(inputs["conv2_b"], np.float32),
    }
    x = np.asarray(inputs["x"], np.float32).reshape(-1, C, HW)
    in_maps = []
    for i in range(n_cores):
        m = dict(shared)
        m["x"] = np.ascontiguousarray(x[i * b_loc:(i + 1) * b_loc])
        in_maps.append(m)
    return in_maps


def kernel(**inputs):
    from concourse import bass_utils
    b_loc = B_GLOBAL // N_CORES
    if "nc" not in _CACHE:
        _CACHE["nc"] = build(N_CORES, b_loc, B_GLOBAL)
    nc = _CACHE["nc"]
    in_maps = _prep_in_maps(inputs, N_CORES, b_loc)
    res = bass_utils.run_bass_kernel_spmd(nc, in_maps, core_ids=list(range(N_CORES)))
    y = np.concatenate([res.results[i]["out"] for i in range(N_CORES)], axis=0)
    return np.ascontiguousarray(y).reshape(B_GLOBAL, C, 32, 32)


# revision 9
# speedup vs baseline: 1.1810x; 1.1810x over previous
"""Fused AttentionLocal kernel for 8 Trainium2 NeuronCores.

Pipeline per batch element b (data-parallel over batch):
  h  = conv7x7_dil2(x)                       [256, 32, 32]
  k  = softmax_ch(BN2(conv1x1(BN1(h))))      [1024, 32, 32]
  y[p, c] = sum_n k[n, p] * x[c, n] / sum_n k[n, p]   (n, p = flattened 32x32)

BN folding (training-mode BN, global batch stats):
  BN1 folds into conv2's weights:   k_raw = (W2 * a1) @ h_raw + cst
  BN2+softmax folds into the exp:   exp(a2 * k_raw + bias2)
  Global stats come from ONE AllReduce of [sum(h), sum(h^2), G = h @ h^T];
  E[k^2] is recovered analytically from the Gram matrix G.
"""

import itertools

import numpy as np

import concourse.bass as bass
import concourse.tile as tile
from concourse import bacc, mybir
from concourse.masks import make_identity

F32 = mybir.dt.float32
F32R = mybir.dt.float32r
BF16 = mybir.dt.bfloat16
AF = mybir.ActivationFunctionType
ALU = mybir.AluOpType
AX = mybir.AxisListType
EPS = 1e-5

N_CORES = 8
B_GLOBAL = 64
C = 256
HW = 1024
W2OUT = 1024

# tap order: (3,3) first so the start=True matmul covers the full PSUM bank
TAPS_ALL = [(3, 3)] + [t for t in itertools.product(range(7), range(7)) if t != (3, 3)]
TAP_GROUPS = []
_i = 0
for _g in (10, 10, 10, 10, 9):
    TAP_GROUPS.append(list(range(_i, _i + _g)))
    _i += _g


def build_body(tc, aps, n_cores, b_loc, total_batch):
    nc = tc.nc
    P_TOT = float(total_batch * HW)
    x_ap = aps["x"]
    xbf_ap = aps["xbf"]
    w1t_ap = aps["w1t"]
    w2t_ap = aps["w2t"]
    out_ap = aps["out"]

    sbsz = min(3, b_loc)
    n_sb = (b_loc + sbsz - 1) // sbsz

    import contextlib
    ctx = contextlib.ExitStack()
    with ctx:
        persist = ctx.enter_context(tc.tile_pool(name="persist", bufs=1))
        dram = ctx.enter_context(tc.tile_pool(name="dram", bufs=1, space="DRAM"))

        # ---------------- prologue: constants + params ----------------
        ident = persist.tile([128, 128], F32, tag="ident", name="ident")
        make_identity(nc, ident[:])
        ones_f32 = persist.tile([128, 2], F32, tag="ones_f32", name="ones_f32")
        nc.gpsimd.memset(ones_f32[:], 1.0)
        ones_col = persist.tile([128, 1], F32R, tag="ones_col", name="ones_col")
        nc.vector.tensor_copy(ones_col[:], ones_f32[:, 0:1])
        zero192 = persist.tile([128, 192], F32, tag="zero192", name="zero192")
        nc.gpsimd.memset(zero192[:], 0.0)

        def row_tile(name, src_1d, n):
            t = persist.tile([1, n], F32, tag=name, name=name)
            nc.sync.dma_start(t[:], src_1d.rearrange("(o n) -> o n", o=1))
            return t

        g1row = row_tile("g1row", aps["bn1g"], C)
        b1row = row_tile("b1row", aps["bn1b"], C)
        g2row = row_tile("g2row", aps["bn2g"], W2OUT)
        b2row = row_tile("b2row", aps["bn2b"], W2OUT)
        cb2row = row_tile("cb2row", aps["cb2"], W2OUT)

        w2t_t = []
        for cc in range(2):
            t = persist.tile([128, W2OUT], F32R, tag=f"w2t{cc}", name=f"w2t{cc}")
            nc.sync.dma_start(t[:], w2t_ap[cc * 128:(cc + 1) * 128, :].bitcast(F32R))
            w2t_t.append(t)

        h_tiles = {}
        for b in range(b_loc):
            for oc in range(2):
                h_tiles[(b, oc)] = persist.tile([128, HW], F32R, tag=f"h{b}_{oc}", name=f"h{b}_{oc}")

        s_acc = [persist.tile([128, b_loc], F32, tag=f"sacc{oc}", name=f"sacc{oc}") for oc in range(2)]
        q_acc = [persist.tile([128, b_loc], F32, tag=f"qacc{oc}", name=f"qacc{oc}") for oc in range(2)]
        s_col = [persist.tile([128, 1], F32, tag=f"scol{oc}", name=f"scol{oc}") for oc in range(2)]
        q_col = [persist.tile([128, 1], F32, tag=f"qcol{oc}", name=f"qcol{oc}") for oc in range(2)]
        bn1pp = [persist.tile([128, 3], F32R, tag=f"bn1pp{oc}", name=f"bn1pp{oc}") for oc in range(2)]
        bn2pp = persist.tile([128, 16], F32, tag="bn2pp", name="bn2pp")

        stats_in = dram.tile([C + 2, C], F32, tag="stats_in", name="stats_in")
        stats_out = dram.tile([C + 2, C], F32, tag="stats_out", name="stats_out")
        bn1_bounce = dram.tile([2, C], F32, tag="bn1_bounce", name="bn1_bounce")
        bn2_bounce = dram.tile([2, W2OUT], F32, tag="bn2_bounce", name="bn2_bounce")

        # ---------------- phase 1: conv1 (dilated 7x7 as 49 shifted matmuls) ----
        with tc.tile_pool(name="convsb", bufs=1) as convsb, \
             tc.tile_pool(name="convps", bufs=8, space="PSUM") as convps:
            for sb in range(n_sb):
                bls = list(range(sb * sbsz, min((sb + 1) * sbsz, b_loc)))
                cps = {}
                for b in bls:
                    for cc in range(2):
                        cp = convsb.tile([128, 32 * 44], BF16, tag=f"colpad{(b % sbsz) * 2 + cc}", name=f"colpad{(b % sbsz) * 2 + cc}", bufs=2)
                        v = cp[:].rearrange("p (r c) -> p r c", c=44)
                        zsrc = zero192[:].rearrange("p (r c) -> p r c", c=6)
                        nc.vector.tensor_copy(v[:, :, 0:6], zsrc)
                        nc.vector.tensor_copy(v[:, :, 38:44], zsrc)
                        nc.gpsimd.dma_start(
                            v[:, :, 6:38],
                            xbf_ap[b, cc * 128:(cc + 1) * 128, :]
                            .rearrange("p (r c) -> p r c", c=32))
                        cps[(b, cc)] = v
                for co in range(2):
                    pss = {}
                    for b in bls:
                        for hf in range(2):
                            pss[(b, hf)] = convps.tile([128, 512], F32, tag="convps", name="convps")
                    for gi, group in enumerate(TAP_GROUPS):
                        g0 = group[0]
                        wts = []
                        for cc in range(2):
                            wt_ = convsb.tile([128, len(group) * 128], BF16, tag=f"w1c{cc}", name=f"w1c{cc}", bufs=3)
                            nc.sync.dma_start(
                                wt_[:].rearrange("p (t k) -> p t k", k=128),
                                w1t_ap[g0:g0 + len(group),
                                       cc * 128:(cc + 1) * 128,
                                       co * 128:(co + 1) * 128]
                                .rearrange("t p k -> p t k"))
                            wts.append(wt_)
                        for b in bls:
                            for hf in range(2):
                                for ti, tap in enumerate(group):
                                    kh, kw = TAPS_ALL[tap]
                                    dy, dx = 2 * kh - 6, 2 * kw - 6
                                    r0 = max(hf * 16, -dy)
                                    r1 = min(hf * 16 + 16, 32 - dy)
                                    if r1 <= r0:
                                        continue
                                    out_v = pss[(b, hf)][:].rearrange(
                                        "p (r c) -> p r c", c=32)[:, r0 - hf * 16:r1 - hf * 16, :]
                                    for cc in range(2):
                                        first = (gi == 0 and ti == 0 and cc == 0)
                                        last = (gi == len(TAP_GROUPS) - 1
                                                and ti == len(group) - 1 and cc == 1)
                                        nc.tensor.matmul(
                                            out_v,
                                            wts[cc][:, ti * 128:(ti + 1) * 128],
                                            cps[(b, cc)][:, r0 + dy:r1 + dy, 6 + dx:6 + dx + 32],
                                            start=first, stop=last, skip_group_check=True)
                    for b in bls:
                        for hf in range(2):
                            dst = h_tiles[(b, co)][:, hf * 512:(hf + 1) * 512]
                            if hf == 0:
                                nc.vector.tensor_copy(dst, pss[(b, hf)][:])
                            else:
                                nc.scalar.copy(dst, pss[(b, hf)][:])

        # ---------------- phase 2: stats + Gram + allreduce ---------------------
        if True:
            with tc.tile_pool(name="gsb", bufs=1) as gsb_pool, \
                 tc.tile_pool(name="gscr", bufs=2) as gscr, \
                 tc.tile_pool(name="hTp", bufs=3) as hTp, \
                 tc.tile_pool(name="tps", bufs=4, space="PSUM") as tps, \
                 tc.tile_pool(name="gps", bufs=1, space="PSUM") as gps_pool:

                # per-channel sum and sum-of-squares of h
                for b in range(b_loc):
                    for oc in range(2):
                        nc.vector.reduce_sum(
                            s_acc[oc][:, b:b + 1], h_tiles[(b, oc)][:].bitcast(F32), axis=AX.X)
                        scr = gscr.tile([128, HW], F32, tag="ttr", name="ttr")
                        nc.scalar.activation(
                            scr[:], h_tiles[(b, oc)][:].bitcast(F32), AF.Square,
                            accum_out=q_acc[oc][:, b:b + 1])
                for oc in range(2):
                    nc.vector.reduce_sum(s_col[oc][:], s_acc[oc][:], axis=AX.X)
                    nc.vector.reduce_sum(q_col[oc][:], q_acc[oc][:], axis=AX.X)

                # G = h @ h^T via PE transposes of h
                gps = [gps_pool.tile([128, C], F32, tag=f"gps{oc}", name=f"gps{oc}") for oc in range(2)]
                for b in range(b_loc):
                    for j in range(8):
                        hT = hTp.tile([128, C], F32R, tag="hT", name="hT")
                        for oc in range(2):
                            tp = tps.tile([128, 128], F32, tag="tp", name="tp")
                            nc.tensor.matmul(
                                tp[:], h_tiles[(b, oc)][:, j * 128:(j + 1) * 128].bitcast(F32),
                                ident[:], is_transpose=True)
                            nc.vector.tensor_copy(hT[:, oc * 128:(oc + 1) * 128], tp[:])
                        for oc in range(2):
                            first = (b == 0 and j == 0)
                            last = (b == b_loc - 1 and j == 7)
                            nc.tensor.matmul(
                                gps[oc][:], hT[:, oc * 128:(oc + 1) * 128], hT[:],
                                start=first, stop=last, skip_group_check=True)
                for oc in range(2):
                    gsb = gsb_pool.tile([128, C], F32, tag=f"gsb{oc}", name=f"gsb{oc}")
                    nc.vector.tensor_copy(gsb[:], gps[oc][:])
                    nc.sync.dma_start(stats_in[oc * 128:(oc + 1) * 128, :], gsb[:])
                    nc.sync.dma_start(
                        stats_in[C:C + 1, oc * 128:(oc + 1) * 128].rearrange("o p -> p o"),
                        s_col[oc][:])
                    nc.sync.dma_start(
                        stats_in[C + 1:C + 2, oc * 128:(oc + 1) * 128].rearrange("o p -> p o"),
                        q_col[oc][:])

                nc.gpsimd.collective_compute(
                    "AllReduce", ALU.add,
                    replica_groups=[list(range(n_cores))],
                    ins=[stats_in.opt()],
                    outs=[stats_out.opt()])

            # ---------------- phase 3: BN constants from global stats ----------
            with tc.tile_pool(name="rows", bufs=1) as rows, \
                 tc.tile_pool(name="prodp", bufs=2) as prodp, \
                 tc.tile_pool(name="rowps", bufs=2, space="PSUM") as rowps, \
                 tc.tile_pool(name="m1ps", bufs=2, space="PSUM") as m1ps:

                def rt(name, n=C):
                    return rows.tile([1, n], F32, tag=name, name=name)

                g_glob = []
                for oc in range(2):
                    gg = rows.tile([128, C], F32R, tag=f"gglob{oc}", name=f"gglob{oc}")
                    nc.sync.dma_start(gg[:], stats_out[oc * 128:(oc + 1) * 128, :].bitcast(F32R))
                    g_glob.append(gg)
                s_row = rt("s_row")
                nc.sync.dma_start(s_row[:], stats_out[C:C + 1, :])
                q_row = rt("q_row")
                nc.sync.dma_start(q_row[:], stats_out[C + 1:C + 2, :])

                meanh = rt("meanh")
                nc.vector.tensor_scalar_mul(meanh[:], s_row[:], 1.0 / P_TOT)
                msq = rt("msq")
                nc.vector.tensor_mul(msq[:], meanh[:], meanh[:])
                var1 = rt("var1")
                nc.vector.tensor_scalar_mul(var1[:], q_row[:], 1.0 / P_TOT)
                nc.vector.tensor_sub(var1[:], var1[:], msq[:])
                nc.vector.tensor_scalar_add(var1[:], var1[:], EPS)
                rec1 = rt("rec1")
                nc.vector.reciprocal(rec1[:], var1[:])
                rsq1 = rt("rsq1")
                nc.scalar.activation(rsq1[:], rec1[:], AF.Sqrt)
                a1row = rt("a1row")
                nc.vector.tensor_mul(a1row[:], rsq1[:], g1row[:])
                tmp1 = rt("tmp1")
                nc.vector.tensor_mul(tmp1[:], a1row[:], meanh[:])
                c1srow = rt("c1srow")
                nc.vector.tensor_sub(c1srow[:], b1row[:], tmp1[:])

                nc.sync.dma_start(bn1_bounce[0:1, :], a1row[:])
                nc.sync.dma_start(bn1_bounce[1:2, :], c1srow[:])
                for oc in range(2):
                    nc.sync.dma_start(
                        bn1pp[oc][:, 0:2],
                        bn1_bounce[:, oc * 128:(oc + 1) * 128]
                        .rearrange("r p -> p r").bitcast(F32R))
                    nc.sync.dma_start(
                        bn1pp[oc][:, 2:3],
                        stats_out[C:C + 1, oc * 128:(oc + 1) * 128]
                        .rearrange("o p -> p o").bitcast(F32R))

                # cst[n] = sum_c W2[n,c] * c1s[c] + conv2_b[n]  (unscaled W2)
                cstrow = rt("cstrow", W2OUT)
                for nh in range(2):
                    cp_ = rowps.tile([1, 512], F32, tag="rowps", name="rowps")
                    for oc in range(2):
                        nc.tensor.matmul(
                            cp_[:], bn1pp[oc][:, 1:2], w2t_t[oc][:, nh * 512:(nh + 1) * 512],
                            start=(oc == 0), stop=(oc == 1), skip_group_check=True)
                    nc.vector.tensor_add(
                        cstrow[:, nh * 512:(nh + 1) * 512], cp_[0:1, :],
                        cb2row[:, nh * 512:(nh + 1) * 512])

                # scale W2T in place by a1 (per input channel)
                for oc in range(2):
                    nc.vector.tensor_scalar_mul(
                        w2t_t[oc][:], w2t_t[oc][:], bn1pp[oc][:, 0:1].bitcast(F32))

                # r1[n] = sum_c W2'[n,c] * s[c]  (scaled W2, unscaled s)
                r1row = rt("r1row", W2OUT)
                for nh in range(2):
                    rp_ = rowps.tile([1, 512], F32, tag="rowps", name="rowps")
                    for oc in range(2):
                        nc.tensor.matmul(
                            rp_[:], bn1pp[oc][:, 2:3], w2t_t[oc][:, nh * 512:(nh + 1) * 512],
                            start=(oc == 0), stop=(oc == 1), skip_group_check=True)
                    nc.vector.tensor_copy(r1row[:, nh * 512:(nh + 1) * 512], rp_[0:1, :])

                # M1 = G @ W2'^T ; e[n] = sum_c W2'[n,c] * M1[c,n]
                M1 = [rows.tile([128, W2OUT], F32R, tag=f"M1_{oc}", name=f"M1_{oc}") for oc in range(2)]
                for occ in range(2):
                    for nh in range(2):
                        mp = m1ps.tile([128, 512], F32, tag="m1ps", name="m1ps")
                        for dd in range(2):
                            nc.tensor.matmul(
                                mp[:], g_glob[dd][:, occ * 128:(occ + 1) * 128],
                                w2t_t[dd][:, nh * 512:(nh + 1) * 512],
                                start=(dd == 0), stop=(dd == 1), skip_group_check=True)
                        nc.vector.tensor_copy(M1[occ][:, nh * 512:(nh + 1) * 512], mp[:])
                erow = rt("erow", W2OUT)
                prods = []
                for oc in range(2):
                    pr = prodp.tile([128, W2OUT], F32R, tag="prod", name="prod")
                    nc.vector.tensor_mul(pr[:], w2t_t[oc][:].bitcast(F32), M1[oc][:].bitcast(F32))
                    prods.append(pr)
                for nh in range(2):
                    ep_ = rowps.tile([1, 512], F32, tag="rowps", name="rowps")
                    for oc in range(2):
                        nc.tensor.matmul(
                            ep_[:], ones_col[:], prods[oc][:, nh * 512:(nh + 1) * 512],
                            start=(oc == 0), stop=(oc == 1), skip_group_check=True)
                    nc.vector.tensor_copy(erow[:, nh * 512:(nh + 1) * 512], ep_[0:1, :])

                # BN2 rows
                mkrow = rt("mkrow", W2OUT)
                nc.vector.tensor_scalar_mul(mkrow[:], r1row[:], 1.0 / P_TOT)
                nc.vector.tensor_add(mkrow[:], mkrow[:], cstrow[:])
                t1 = rt("t1", W2OUT)
                nc.vector.tensor_mul(t1[:], cstrow[:], r1row[:])
                nc.vector.tensor_scalar_mul(t1[:], t1[:], 2.0 / P_TOT)
                t2 = rt("t2", W2OUT)
                nc.vector.tensor_mul(t2[:], cstrow[:], cstrow[:])
                ek2 = rt("ek2", W2OUT)
                nc.vector.tensor_scalar_mul(ek2[:], erow[:], 1.0 / P_TOT)
                nc.vector.tensor_add(ek2[:], ek2[:], t1[:])
                nc.vector.tensor_add(ek2[:], ek2[:], t2[:])
                mks = rt("mks", W2OUT)
                nc.vector.tensor_mul(mks[:], mkrow[:], mkrow[:])
                var2 = rt("var2", W2OUT)
                nc.vector.tensor_sub(var2[:], ek2[:], mks[:])
                nc.vector.tensor_scalar_add(var2[:], var2[:], EPS)
                rec2 = rt("rec2", W2OUT)
                nc.vector.reciprocal(rec2[:], var2[:])
                rsq2 = rt("rsq2", W2OUT)
                nc.scalar.activation(rsq2[:], rec2[:], AF.Sqrt)
                a2row = rt("a2row", W2OUT)
                nc.vector.tensor_mul(a2row[:], rsq2[:], g2row[:])
                t3 = rt("t3", W2OUT)
                nc.vector.tensor_scalar_mul(t3[:], r1row[:], 1.0 / P_TOT)
                nc.vector.tensor_mul(t3[:], a2row[:], t3[:])
                bias2row = rt("bias2row", W2OUT)
                nc.vector.tensor_sub(bias2row[:], b2row[:], t3[:])

                nc.sync.dma_start(bn2_bounce[0:1, :], a2row[:])
                nc.sync.dma_start(bn2_bounce[1:2, :], bias2row[:])
                nc.sync.dma_start(
                    bn2pp[:],
                    bn2_bounce[:].rearrange("w (k p) -> p (w k)", p=128))

            # ---------------- phase 4: x transposes + conv2 + exp + attention ---
            with tc.tile_pool(name="xta", bufs=1) as xta_pool, \
                 tc.tile_pool(name="xinp", bufs=2) as xinp, \
                 tc.tile_pool(name="kexp", bufs=12) as kexp_pool, \
                 tc.tile_pool(name="outp", bufs=4) as outp, \
                 tc.tile_pool(name="recp", bufs=4) as recp, \
                 tc.tile_pool(name="tps2", bufs=2, space="PSUM") as tps2, \
                 tc.tile_pool(name="c2ps", bufs=3, space="PSUM") as c2ps, \
                 tc.tile_pool(name="aps", bufs=3, space="PSUM") as aps_pool:
                # xTa[b][j] = [x[b]^T slice | ones] for the attention matmul
                xta = {}
                for b in range(b_loc):
                    xins = []
                    for cc in range(2):
                        xin = xinp.tile([128, HW], F32R, tag=f"xin{cc}", name=f"xin{cc}")
                        nc.sync.dma_start(
                            xin[:], x_ap[b, cc * 128:(cc + 1) * 128, :].bitcast(F32R))
                        xins.append(xin)
                    for j in range(8):
                        xt_ = xta_pool.tile([128, 258], F32R, tag=f"xta{b}_{j}", name=f"xta{b}_{j}")
                        for cc in range(2):
                            tp = tps2.tile([128, 128], F32, tag="tp2", name="tp2")
                            nc.tensor.matmul(
                                tp[:], xins[cc][:, j * 128:(j + 1) * 128].bitcast(F32),
                                ident[:], is_transpose=True)
                            nc.vector.tensor_copy(xt_[:, cc * 128:(cc + 1) * 128], tp[:])
                        nc.vector.tensor_copy(xt_[:, 256:258], ones_f32[:])
                        xta[(b, j)] = xt_
                for b in range(b_loc):
                    for hf in range(2):
                        ke = []
                        for j in range(8):
                            cp_ = c2ps.tile([128, 512], F32, tag="c2ps", name="c2ps")
                            for cc in range(2):
                                nc.tensor.matmul(
                                    cp_[:], w2t_t[cc][:, j * 128:(j + 1) * 128],
                                    h_tiles[(b, cc)][:, hf * 512:(hf + 1) * 512],
                                    start=(cc == 0), stop=(cc == 1), skip_group_check=True)
                            ket = kexp_pool.tile([128, 512], F32R, tag="ke", name="ke")
                            nc.scalar.activation(
                                ket[:], cp_[:], AF.Exp,
                                bias=bn2pp[:, 8 + j:9 + j], scale=bn2pp[:, j:j + 1])
                            ke.append(ket)
                        for pc in range(4):
                            ap_ = aps_pool.tile([128, 258], F32, tag="aps", name="aps")
                            for j in range(8):
                                nc.tensor.matmul(
                                    ap_[:], ke[j][:, pc * 128:(pc + 1) * 128], xta[(b, j)][:],
                                    start=(j == 0), stop=(j == 7), skip_group_check=True)
                            rec = recp.tile([128, 1], F32, tag="rec", name="rec")
                            nc.vector.reciprocal(rec[:], ap_[:, 256:257])
                            osb = outp.tile([128, C], F32, tag="osb", name="osb")
                            nc.vector.tensor_scalar_mul(osb[:], ap_[:, 0:256], rec[:])
                            r0 = hf * 512 + pc * 128
                            nc.sync.dma_start(out_ap[b, r0:r0 + 128, :], osb[:])


def build(n_cores=N_CORES, b_loc=B_GLOBAL // N_CORES, total_batch=B_GLOBAL):
    nc = bacc.Bacc("TRN2", target_bir_lowering=False, debug=False, num_devices=n_cores)
    aps = {
        "x": nc.dram_tensor("x", [b_loc, C, HW], F32, kind="ExternalInput").ap(),
        "xbf": nc.dram_tensor("xbf", [b_loc, C, HW], mybir.dt.bfloat16, kind="ExternalInput").ap(),
        "w1t": nc.dram_tensor("w1t", [49, C, C], mybir.dt.bfloat16, kind="ExternalInput").ap(),
        "w2t": nc.dram_tensor("w2t", [C, W2OUT], F32, kind="ExternalInput").ap(),
        "bn1g": nc.dram_tensor("bn1g", [C], F32, kind="ExternalInput").ap(),
        "bn1b": nc.dram_tensor("bn1b", [C], F32, kind="ExternalInput").ap(),
        "bn2g": nc.dram_tensor("bn2g", [W2OUT], F32, kind="ExternalInput").ap(),
        "bn2b": nc.dram_tensor("bn2b", [W2OUT], F32, kind="ExternalInput").ap(),
        "cb2": nc.dram_tensor("cb2", [W2OUT], F32, kind="ExternalInput").ap(),
        "out": nc.dram_tensor("out", [b_loc, HW, C], F32, kind="ExternalOutput").ap(),
    }
    with tile.TileContext(nc) as tc:
        build_body(tc, aps, n_cores, b_loc, total_batch)
    nc.compile()
    return nc


_CACHE = {}


def _prep_in_maps(inputs, n_cores, b_loc):
    import ml_dtypes
    perm = [kh * 7 + kw for (kh, kw) in TAPS_ALL]
    import ml_dtypes
    w1t = np.ascontiguousarray(
        np.asarray(inputs["conv1_w"], np.float32).transpose(2, 3, 1, 0).reshape(49, C, C)[perm]
    ).astype(ml_dtypes.bfloat16)
    w2t = np.ascontiguousarray(np.asarray(inputs["conv2_w"], np.float32)[:, :, 0, 0].T)
    shared = {
        "w1t": w1t,
        "w2t": w2t,
        "bn1g": np.asarray(inputs["bn1_g"], np.float32),
        "bn1b": np.asarray(inputs["bn1_b"], np.float32),
        "bn2g": np.asarray(inputs["bn2_g"], np.float32),
        "bn2b": np.asarray(inputs["bn2_b"], np.float32),
        "cb2": np.asarray(inputs["conv2_b"], np.float32),
    }
    x = np.asarray(inputs["x"], np.float32).reshape(-1, C, HW)
    in_maps = []
    for i in range(n_cores):
        m = dict(shared)
        xs = np.ascontiguousarray(x[i * b_loc:(i + 1) * b_loc])
        m["x"] = xs
        m["xbf"] = xs.astype(ml_dtypes.bfloat16)
        in_maps.append(m)
    return in_maps


def kernel(**inputs):
    from concourse import bass_utils
    b_loc = B_GLOBAL // N_CORES
    if "nc" not in _CACHE:
        _CACHE["nc"] = build(N_CORES, b_loc, B_GLOBAL)
    nc = _CACHE["nc"]
    in_maps = _prep_in_maps(inputs, N_CORES, b_loc)
    res = bass_utils.run_bass_kernel_spmd(nc, in_maps, core_ids=list(range(N_CORES)))
    y = np.concatenate([res.results[i]["out"] for i in range(N_CORES)], axis=0)
    return np.ascontiguousarray(y).reshape(B_GLOBAL, C, 32, 32)


# revision 10
# speedup vs baseline: 1.1928x; 1.0100x over previous
"""Fused AttentionLocal kernel for 8 Trainium2 NeuronCores.

Pipeline per batch element b (data-parallel over batch):
  h  = conv7x7_dil2(x)                       [256, 32, 32]
  k  = softmax_ch(BN2(conv1x1(BN1(h))))      [1024, 32, 32]
  y[p, c] = sum_n k[n, p] * x[c, n] / sum_n k[n, p]   (n, p = flattened 32x32)

BN folding (training-mode BN, global batch stats):
  BN1 folds into conv2's weights:   k_raw = (W2 * a1) @ h_raw + cst
  BN2+softmax folds into the exp:   exp(a2 * k_raw + bias2)
  Global stats come from ONE AllReduce of [sum(h), sum(h^2), G = h @ h^T];
  E[k^2] is recovered analytically from the Gram matrix G.
"""

import itertools

import numpy as np

import concourse.bass as bass
import concourse.tile as tile
from concourse import bacc, mybir
from concourse.masks import make_identity

F32 = mybir.dt.float32
F32R = mybir.dt.float32r
BF16 = mybir.dt.bfloat16
AF = mybir.ActivationFunctionType
ALU = mybir.AluOpType
AX = mybir.AxisListType
EPS = 1e-5

N_CORES = 8
B_GLOBAL = 64
C = 256
HW = 1024
W2OUT = 1024

# tap order: (3,3) first so the start=True matmul covers the full PSUM bank
TAPS_ALL = [(3, 3)] + [t for t in itertools.product(range(7), range(7)) if t != (3, 3)]
TAP_GROUPS = []
_i = 0
for _g in (10, 10, 10, 10, 9):
    TAP_GROUPS.append(list(range(_i, _i + _g)))
    _i += _g


def build_body(tc, aps, n_cores, b_loc, total_batch):
    nc = tc.nc
    P_TOT = float(total_batch * HW)
    x_ap = aps["x"]
    xbf_ap = aps["xbf"]
    w1t_ap = aps["w1t"]
    w2t_ap = aps["w2t"]
    out_ap = aps["out"]

    sbsz = min(3, b_loc)
    n_sb = (b_loc + sbsz - 1) // sbsz

    import contextlib
    ctx = contextlib.ExitStack()
    with ctx:
        persist = ctx.enter_context(tc.tile_pool(name="persist", bufs=1))
        dram = ctx.enter_context(tc.tile_pool(name="dram", bufs=1, space="DRAM"))

        # ---------------- prologue: constants + params ----------------
        ident = persist.tile([128, 128], F32, tag="ident", name="ident")
        make_identity(nc, ident[:])
        identb = persist.tile([128, 128], BF16, tag="identb", name="identb")
        make_identity(nc, identb[:])
        ones_f32 = persist.tile([128, 2], F32, tag="ones_f32", name="ones_f32")
        nc.gpsimd.memset(ones_f32[:], 1.0)
        ones_col = persist.tile([128, 1], F32R, tag="ones_col", name="ones_col")
        nc.vector.tensor_copy(ones_col[:], ones_f32[:, 0:1])
        zero192 = persist.tile([128, 192], F32, tag="zero192", name="zero192")
        nc.gpsimd.memset(zero192[:], 0.0)

        def row_tile(name, src_1d, n):
            t = persist.tile([1, n], F32, tag=name, name=name)
            nc.sync.dma_start(t[:], src_1d.rearrange("(o n) -> o n", o=1))
            return t

        g1row = row_tile("g1row", aps["bn1g"], C)
        b1row = row_tile("b1row", aps["bn1b"], C)
        g2row = row_tile("g2row", aps["bn2g"], W2OUT)
        b2row = row_tile("b2row", aps["bn2b"], W2OUT)
        cb2row = row_tile("cb2row", aps["cb2"], W2OUT)

        w2t_t = []
        for cc in range(2):
            t = persist.tile([128, W2OUT], F32R, tag=f"w2t{cc}", name=f"w2t{cc}")
            nc.sync.dma_start(t[:], w2t_ap[cc * 128:(cc + 1) * 128, :].bitcast(F32R))
            w2t_t.append(t)

        h_tiles = {}
        for b in range(b_loc):
            for oc in range(2):
                h_tiles[(b, oc)] = persist.tile([128, HW], BF16, tag=f"h{b}_{oc}", name=f"h{b}_{oc}")

        s_acc = [persist.tile([128, b_loc], F32, tag=f"sacc{oc}", name=f"sacc{oc}") for oc in range(2)]
        q_acc = [persist.tile([128, b_loc], F32, tag=f"qacc{oc}", name=f"qacc{oc}") for oc in range(2)]
        s_col = [persist.tile([128, 1], F32, tag=f"scol{oc}", name=f"scol{oc}") for oc in range(2)]
        q_col = [persist.tile([128, 1], F32, tag=f"qcol{oc}", name=f"qcol{oc}") for oc in range(2)]
        bn1pp = [persist.tile([128, 3], F32R, tag=f"bn1pp{oc}", name=f"bn1pp{oc}") for oc in range(2)]
        bn2pp = persist.tile([128, 16], F32, tag="bn2pp", name="bn2pp")

        stats_in = dram.tile([C + 2, C], F32, tag="stats_in", name="stats_in")
        stats_out = dram.tile([C + 2, C], F32, tag="stats_out", name="stats_out")
        bn1_bounce = dram.tile([2, C], F32, tag="bn1_bounce", name="bn1_bounce")
        bn2_bounce = dram.tile([2, W2OUT], F32, tag="bn2_bounce", name="bn2_bounce")

        # ---------------- phase 1: conv1 (dilated 7x7 as 49 shifted matmuls) ----
        with tc.tile_pool(name="convsb", bufs=1) as convsb, \
             tc.tile_pool(name="convps", bufs=8, space="PSUM") as convps:
            for sb in range(n_sb):
                bls = list(range(sb * sbsz, min((sb + 1) * sbsz, b_loc)))
                cps = {}
                for b in bls:
                    for cc in range(2):
                        cp = convsb.tile([128, 32 * 44], BF16, tag=f"colpad{(b % sbsz) * 2 + cc}", name=f"colpad{(b % sbsz) * 2 + cc}", bufs=2)
                        v = cp[:].rearrange("p (r c) -> p r c", c=44)
                        zsrc = zero192[:].rearrange("p (r c) -> p r c", c=6)
                        nc.vector.tensor_copy(v[:, :, 0:6], zsrc)
                        nc.vector.tensor_copy(v[:, :, 38:44], zsrc)
                        nc.gpsimd.dma_start(
                            v[:, :, 6:38],
                            xbf_ap[b, cc * 128:(cc + 1) * 128, :]
                            .rearrange("p (r c) -> p r c", c=32))
                        cps[(b, cc)] = v
                for co in range(2):
                    pss = {}
                    for b in bls:
                        for hf in range(2):
                            pss[(b, hf)] = convps.tile([128, 512], F32, tag="convps", name="convps")
                    for gi, group in enumerate(TAP_GROUPS):
                        g0 = group[0]
                        wts = []
                        for cc in range(2):
                            wt_ = convsb.tile([128, len(group) * 128], BF16, tag=f"w1c{cc}", name=f"w1c{cc}", bufs=3)
                            nc.sync.dma_start(
                                wt_[:].rearrange("p (t k) -> p t k", k=128),
                                w1t_ap[g0:g0 + len(group),
                                       cc * 128:(cc + 1) * 128,
                                       co * 128:(co + 1) * 128]
                                .rearrange("t p k -> p t k"))
                            wts.append(wt_)
                        for b in bls:
                            for hf in range(2):
                                for ti, tap in enumerate(group):
                                    kh, kw = TAPS_ALL[tap]
                                    dy, dx = 2 * kh - 6, 2 * kw - 6
                                    r0 = max(hf * 16, -dy)
                                    r1 = min(hf * 16 + 16, 32 - dy)
                                    if r1 <= r0:
                                        continue
                                    out_v = pss[(b, hf)][:].rearrange(
                                        "p (r c) -> p r c", c=32)[:, r0 - hf * 16:r1 - hf * 16, :]
                                    for cc in range(2):
                                        first = (gi == 0 and ti == 0 and cc == 0)
                                        last = (gi == len(TAP_GROUPS) - 1
                                                and ti == len(group) - 1 and cc == 1)
                                        nc.tensor.matmul(
                                            out_v,
                                            wts[cc][:, ti * 128:(ti + 1) * 128],
                                            cps[(b, cc)][:, r0 + dy:r1 + dy, 6 + dx:6 + dx + 32],
                                            start=first, stop=last, skip_group_check=True)
                    for b in bls:
                        for hf in range(2):
                            dst = h_tiles[(b, co)][:, hf * 512:(hf + 1) * 512]
                            if hf == 0:
                                nc.vector.tensor_copy(dst, pss[(b, hf)][:])
                            else:
                                nc.scalar.copy(dst, pss[(b, hf)][:])

        # ---------------- phase 2: stats + Gram + allreduce ---------------------
        xta_pool = ctx.enter_context(tc.tile_pool(name="xta", bufs=1))
        xta = {}
        if True:
            with tc.tile_pool(name="gsb", bufs=1) as gsb_pool, \
                 tc.tile_pool(name="gscr", bufs=2) as gscr, \
                 tc.tile_pool(name="hTp", bufs=3) as hTp, \
                 tc.tile_pool(name="tps", bufs=4, space="PSUM") as tps, \
                 tc.tile_pool(name="gps", bufs=1, space="PSUM") as gps_pool:

                # per-channel sum and sum-of-squares of h
                for b in range(b_loc):
                    for oc in range(2):
                        nc.vector.reduce_sum(
                            s_acc[oc][:, b:b + 1], h_tiles[(b, oc)][:], axis=AX.X)
                        scr = gscr.tile([128, HW], F32, tag="ttr", name="ttr")
                        nc.scalar.activation(
                            scr[:], h_tiles[(b, oc)][:], AF.Square,
                            accum_out=q_acc[oc][:, b:b + 1])
                for oc in range(2):
                    nc.vector.reduce_sum(s_col[oc][:], s_acc[oc][:], axis=AX.X)
                    nc.vector.reduce_sum(q_col[oc][:], q_acc[oc][:], axis=AX.X)

                # G = h @ h^T via PE transposes of h
                gps = [gps_pool.tile([128, C], F32, tag=f"gps{oc}", name=f"gps{oc}") for oc in range(2)]
                for b in range(b_loc):
                    for j in range(8):
                        hT = hTp.tile([128, C], BF16, tag="hT", name="hT")
                        for oc in range(2):
                            tp = tps.tile([128, 128], BF16, tag="tp", name="tp")
                            nc.tensor.matmul(
                                tp[:], h_tiles[(b, oc)][:, j * 128:(j + 1) * 128],
                                identb[:], is_transpose=True)
                            nc.vector.tensor_copy(hT[:, oc * 128:(oc + 1) * 128], tp[:])
                        for oc in range(2):
                            first = (b == 0 and j == 0)
                            last = (b == b_loc - 1 and j == 7)
                            nc.tensor.matmul(
                                gps[oc][:], hT[:, oc * 128:(oc + 1) * 128], hT[:],
                                start=first, stop=last, skip_group_check=True)
                for oc in range(2):
                    gsb = gsb_pool.tile([128, C], F32, tag=f"gsb{oc}", name=f"gsb{oc}")
                    nc.vector.tensor_copy(gsb[:], gps[oc][:])
                    nc.sync.dma_start(stats_in[oc * 128:(oc + 1) * 128, :], gsb[:])
                    nc.sync.dma_start(
                        stats_in[C:C + 1, oc * 128:(oc + 1) * 128].rearrange("o p -> p o"),
                        s_col[oc][:])
                    nc.sync.dma_start(
                        stats_in[C + 1:C + 2, oc * 128:(oc + 1) * 128].rearrange("o p -> p o"),
                        q_col[oc][:])

                nc.gpsimd.collective_compute(
                    "AllReduce", ALU.add,
                    replica_groups=[list(range(n_cores))],
                    ins=[stats_in.opt()],
                    outs=[stats_out.opt()])

                # xTa[b][j] = [x[b]^T slice | ones] (bf16) - overlaps the collective
                for b in range(b_loc):
                    xins = []
                    for cc in range(2):
                        xin = hTp.tile([128, HW], F32, tag=f"xin{cc}", name=f"xin{cc}", bufs=2)
                        nc.sync.dma_start(
                            xin[:], x_ap[b, cc * 128:(cc + 1) * 128, :])
                        xins.append(xin)
                    for j in range(8):
                        xt_ = xta_pool.tile([128, 258], BF16, tag=f"xta{b}_{j}", name=f"xta{b}_{j}")
                        for cc in range(2):
                            tpx = tps.tile([128, 128], F32, tag="tpx", name="tpx", bufs=2)
                            nc.tensor.matmul(
                                tpx[:], xins[cc][:, j * 128:(j + 1) * 128],
                                ident[:], is_transpose=True)
                            nc.vector.tensor_copy(xt_[:, cc * 128:(cc + 1) * 128], tpx[:])
                        nc.vector.tensor_copy(xt_[:, 256:258], ones_f32[:])
                        xta[(b, j)] = xt_

            # ---------------- phase 3: BN constants from global stats ----------
            with tc.tile_pool(name="rows", bufs=1) as rows, \
                 tc.tile_pool(name="prodp", bufs=2) as prodp, \
                 tc.tile_pool(name="rowps", bufs=2, space="PSUM") as rowps, \
                 tc.tile_pool(name="m1ps", bufs=2, space="PSUM") as m1ps:

                def rt(name, n=C):
                    return rows.tile([1, n], F32, tag=name, name=name)

                g_glob = []
                for oc in range(2):
                    gg = rows.tile([128, C], F32R, tag=f"gglob{oc}", name=f"gglob{oc}")
                    nc.sync.dma_start(gg[:], stats_out[oc * 128:(oc + 1) * 128, :].bitcast(F32R))
                    g_glob.append(gg)
                s_row = rt("s_row")
                nc.sync.dma_start(s_row[:], stats_out[C:C + 1, :])
                q_row = rt("q_row")
                nc.sync.dma_start(q_row[:], stats_out[C + 1:C + 2, :])

                meanh = rt("meanh")
                nc.vector.tensor_scalar_mul(meanh[:], s_row[:], 1.0 / P_TOT)
                msq = rt("msq")
                nc.vector.tensor_mul(msq[:], meanh[:], meanh[:])
                var1 = rt("var1")
                nc.vector.tensor_scalar_mul(var1[:], q_row[:], 1.0 / P_TOT)
                nc.vector.tensor_sub(var1[:], var1[:], msq[:])
                nc.vector.tensor_scalar_add(var1[:], var1[:], EPS)
                rec1 = rt("rec1")
                nc.vector.reciprocal(rec1[:], var1[:])
                rsq1 = rt("rsq1")
                nc.scalar.activation(rsq1[:], rec1[:], AF.Sqrt)
                a1row = rt("a1row")
                nc.vector.tensor_mul(a1row[:], rsq1[:], g1row[:])
                tmp1 = rt("tmp1")
                nc.vector.tensor_mul(tmp1[:], a1row[:], meanh[:])
                c1srow = rt("c1srow")
                nc.vector.tensor_sub(c1srow[:], b1row[:], tmp1[:])

                nc.sync.dma_start(bn1_bounce[0:1, :], a1row[:])
                nc.sync.dma_start(bn1_bounce[1:2, :], c1srow[:])
                for oc in range(2):
                    nc.sync.dma_start(
                        bn1pp[oc][:, 0:2],
                        bn1_bounce[:, oc * 128:(oc + 1) * 128]
                        .rearrange("r p -> p r").bitcast(F32R))
                    nc.sync.dma_start(
                        bn1pp[oc][:, 2:3],
                        stats_out[C:C + 1, oc * 128:(oc + 1) * 128]
                        .rearrange("o p -> p o").bitcast(F32R))

                # cst[n] = sum_c W2[n,c] * c1s[c] + conv2_b[n]  (unscaled W2)
                cstrow = rt("cstrow", W2OUT)
                for nh in range(2):
                    cp_ = rowps.tile([1, 512], F32, tag="rowps", name="rowps")
                    for oc in range(2):
                        nc.tensor.matmul(
                            cp_[:], bn1pp[oc][:, 1:2], w2t_t[oc][:, nh * 512:(nh + 1) * 512],
                            start=(oc == 0), stop=(oc == 1), skip_group_check=True)
                    nc.vector.tensor_add(
                        cstrow[:, nh * 512:(nh + 1) * 512], cp_[0:1, :],
                        cb2row[:, nh * 512:(nh + 1) * 512])

                # scale W2T in place by a1 (per input channel)
                for oc in range(2):
                    nc.vector.tensor_scalar_mul(
                        w2t_t[oc][:], w2t_t[oc][:], bn1pp[oc][:, 0:1].bitcast(F32))

                # r1[n] = sum_c W2'[n,c] * s[c]  (scaled W2, unscaled s)
                r1row = rt("r1row", W2OUT)
                for nh in range(2):
                    rp_ = rowps.tile([1, 512], F32, tag="rowps", name="rowps")
                    for oc in range(2):
                        nc.tensor.matmul(
                            rp_[:], bn1pp[oc][:, 2:3], w2t_t[oc][:, nh * 512:(nh + 1) * 512],
                            start=(oc == 0), stop=(oc == 1), skip_group_check=True)
                    nc.vector.tensor_copy(r1row[:, nh * 512:(nh + 1) * 512], rp_[0:1, :])

                # M1 = G @ W2'^T ; e[n] = sum_c W2'[n,c] * M1[c,n]
                M1 = [rows.tile([128, W2OUT], F32R, tag=f"M1_{oc}", name=f"M1_{oc}") for oc in range(2)]
                for occ in range(2):
                    for nh in range(2):
                        mp = m1ps.tile([128, 512], F32, tag="m1ps", name="m1ps")
                        for dd in range(2):
                            nc.tensor.matmul(
                                mp[:], g_glob[dd][:, occ * 128:(occ + 1) * 128],
                                w2t_t[dd][:, nh * 512:(nh + 1) * 512],
                                start=(dd == 0), stop=(dd == 1), skip_group_check=True)
                        nc.vector.tensor_copy(M1[occ][:, nh * 512:(nh + 1) * 512], mp[:])
                erow = rt("erow", W2OUT)
                prods = []
                for oc in range(2):
                    pr = prodp.tile([128, W2OUT], F32R, tag="prod", name="prod")
                    nc.vector.tensor_mul(pr[:], w2t_t[oc][:].bitcast(F32), M1[oc][:].bitcast(F32))
                    prods.append(pr)
                for nh in range(2):
                    ep_ = rowps.tile([1, 512], F32, tag="rowps", name="rowps")
                    for oc in range(2):
                        nc.tensor.matmul(
                            ep_[:], ones_col[:], prods[oc][:, nh * 512:(nh + 1) * 512],
                            start=(oc == 0), stop=(oc == 1), skip_group_check=True)
                    nc.vector.tensor_copy(erow[:, nh * 512:(nh + 1) * 512], ep_[0:1, :])

                # BN2 rows
                mkrow = rt("mkrow", W2OUT)
                nc.vector.tensor_scalar_mul(mkrow[:], r1row[:], 1.0 / P_TOT)
                nc.vector.tensor_add(mkrow[:], mkrow[:], cstrow[:])
                t1 = rt("t1", W2OUT)
                nc.vector.tensor_mul(t1[:], cstrow[:], r1row[:])
                nc.vector.tensor_scalar_mul(t1[:], t1[:], 2.0 / P_TOT)
                t2 = rt("t2", W2OUT)
                nc.vector.tensor_mul(t2[:], cstrow[:], cstrow[:])
                ek2 = rt("ek2", W2OUT)
                nc.vector.tensor_scalar_mul(ek2[:], erow[:], 1.0 / P_TOT)
                nc.vector.tensor_add(ek2[:], ek2[:], t1[:])
                nc.vector.tensor_add(ek2[:], ek2[:], t2[:])
                mks = rt("mks", W2OUT)
                nc.vector.tensor_mul(mks[:], mkrow[:], mkrow[:])
                var2 = rt("var2", W2OUT)
                nc.vector.tensor_sub(var2[:], ek2[:], mks[:])
                nc.vector.tensor_scalar_add(var2[:], var2[:], EPS)
                rec2 = rt("rec2", W2OUT)
                nc.vector.reciprocal(rec2[:], var2[:])
                rsq2 = rt("rsq2", W2OUT)
                nc.scalar.activation(rsq2[:], rec2[:], AF.Sqrt)
                a2row = rt("a2row", W2OUT)
                nc.vector.tensor_mul(a2row[:], rsq2[:], g2row[:])
                t3 = rt("t3", W2OUT)
                nc.vector.tensor_scalar_mul(t3[:], r1row[:], 1.0 / P_TOT)
                nc.vector.tensor_mul(t3[:], a2row[:], t3[:])
                bias2row = rt("bias2row", W2OUT)
                nc.vector.tensor_sub(bias2row[:], b2row[:], t3[:])

                nc.sync.dma_start(bn2_bounce[0:1, :], a2row[:])
                nc.sync.dma_start(bn2_bounce[1:2, :], bias2row[:])
                nc.sync.dma_start(
                    bn2pp[:],
                    bn2_bounce[:].rearrange("w (k p) -> p (w k)", p=128))

            # ---------------- phase 4: conv2 + exp + attention ------------------
            with tc.tile_pool(name="w2bp", bufs=1) as w2bp, \
                 tc.tile_pool(name="kexp", bufs=12) as kexp_pool, \
                 tc.tile_pool(name="outp", bufs=4) as outp, \
                 tc.tile_pool(name="recp", bufs=4) as recp, \
                 tc.tile_pool(name="c2ps", bufs=3, space="PSUM") as c2ps, \
                 tc.tile_pool(name="aps", bufs=3, space="PSUM") as aps_pool:
                # bf16 copy of the scaled conv2 weights
                w2tb = []
                for cc in range(2):
                    wb = w2bp.tile([128, W2OUT], BF16, tag=f"w2tb{cc}", name=f"w2tb{cc}")
                    nc.vector.tensor_copy(wb[:], w2t_t[cc][:].bitcast(F32))
                    w2tb.append(wb)
                for b in range(b_loc):
                    for hf in range(2):
                        ke = []
                        for j in range(8):
                            cp_ = c2ps.tile([128, 512], F32, tag="c2ps", name="c2ps")
                            for cc in range(2):
                                nc.tensor.matmul(
                                    cp_[:], w2tb[cc][:, j * 128:(j + 1) * 128],
                                    h_tiles[(b, cc)][:, hf * 512:(hf + 1) * 512],
                                    start=(cc == 0), stop=(cc == 1), skip_group_check=True)
                            ket = kexp_pool.tile([128, 512], BF16, tag="ke", name="ke")
                            nc.scalar.activation(
                                ket[:], cp_[:], AF.Exp,
                                bias=bn2pp[:, 8 + j:9 + j], scale=bn2pp[:, j:j + 1])
                            ke.append(ket)
                        for pc in range(4):
                            ap_ = aps_pool.tile([128, 258], F32, tag="aps", name="aps")
                            for j in range(8):
                                nc.tensor.matmul(
                                    ap_[:], ke[j][:, pc * 128:(pc + 1) * 128], xta[(b, j)][:],
                                    start=(j == 0), stop=(j == 7), skip_group_check=True)
                            rec = recp.tile([128, 1], F32, tag="rec", name="rec")
                            nc.vector.reciprocal(rec[:], ap_[:, 256:257])
                            osb = outp.tile([128, C], F32, tag="osb", name="osb")
                            nc.vector.tensor_scalar_mul(osb[:], ap_[:, 0:256], rec[:])
                            r0 = hf * 512 + pc * 128
                            nc.sync.dma_start(out_ap[b, r0:r0 + 128, :], osb[:])


def build(n_cores=N_CORES, b_loc=B_GLOBAL // N_CORES, total_batch=B_GLOBAL):
    nc = bacc.Bacc("TRN2", target_bir_lowering=False, debug=False, num_devices=n_cores)
    aps = {
        "x": nc.dram_tensor("x", [b_loc, C, HW], F32, kind="ExternalInput").ap(),
        "xbf": nc.dram_tensor("xbf", [b_loc, C, HW], mybir.dt.bfloat16, kind="ExternalInput").ap(),
        "w1t": nc.dram_tensor("w1t", [49, C, C], mybir.dt.bfloat16, kind="ExternalInput").ap(),
        "w2t": nc.dram_tensor("w2t", [C, W2OUT], F32, kind="ExternalInput").ap(),
        "bn1g": nc.dram_tensor("bn1g", [C], F32, kind="ExternalInput").ap(),
        "bn1b": nc.dram_tensor("bn1b", [C], F32, kind="ExternalInput").ap(),
        "bn2g": nc.dram_tensor("bn2g", [W2OUT], F32, kind="ExternalInput").ap(),
        "bn2b": nc.dram_tensor("bn2b", [W2OUT], F32, kind="ExternalInput").ap(),
        "cb2": nc.dram_tensor("cb2", [W2OUT], F32, kind="ExternalInput").ap(),
        "out": nc.dram_tensor("out", [b_loc, HW, C], F32, kind="ExternalOutput").ap(),
    }
    with tile.TileContext(nc) as tc:
        build_body(tc, aps, n_cores, b_loc, total_batch)
    nc.compile()
    return nc


_CACHE = {}


def _prep_in_maps(inputs, n_cores, b_loc):
    import ml_dtypes
    perm = [kh * 7 + kw for (kh, kw) in TAPS_ALL]
    import ml_dtypes
    w1t = np.ascontiguousarray(
        np.asarray(inputs["conv1_w"], np.float32).transpose(2, 3, 1, 0).reshape(49, C, C)[perm]
    ).astype(ml_dtypes.bfloat16)
    w2t = np.ascontiguousarray(np.asarray(inputs["conv2_w"], np.float32)[:, :, 0, 0].T)
    shared = {
        "w1t": w1t,
        "w2t": w2t,
        "bn1g": np.asarray(inputs["bn1_g"], np.float32),
        "bn1b": np.asarray(inputs["bn1_b"], np.float32),
        "bn2g": np.asarray(inputs["bn2_g"], np.float32),
        "bn2b": np.asarray(inputs["bn2_b"], np.float32),
        "cb2": np.asarray(inputs["conv2_b"], np.float32),
    }
    x = np.asarray(inputs["x"], np.float32).reshape(-1, C, HW)
    in_maps = []
    for i in range(n_cores):
        m = dict(shared)
        xs = np.ascontiguousarray(x[i * b_loc:(i + 1) * b_loc])
        m["x"] = xs
        m["xbf"] = xs.astype(ml_dtypes.bfloat16)
        in_maps.append(m)
    return in_maps


def kernel(**inputs):
    from concourse import bass_utils
    b_loc = B_GLOBAL // N_CORES
    if "nc" not in _CACHE:
        _CACHE["nc"] = build(N_CORES, b_loc, B_GLOBAL)
    nc = _CACHE["nc"]
    in_maps = _prep_in_maps(inputs, N_CORES, b_loc)
    res = bass_utils.run_bass_kernel_spmd(nc, in_maps, core_ids=list(range(N_CORES)))
    y = np.concatenate([res.results[i]["out"] for i in range(N_CORES)], axis=0)
    return np.ascontiguousarray(y).reshape(B_GLOBAL, C, 32, 32)
